# revision 9
# baseline (speedup 1.0000x reference)
"""DLoRF low-rank linear kernel for Trainium2 (8 NeuronCores, SPMD).

Computes  out = x @ U @ diag(s * mask) @ V.T  for
  x [8, 2048, 4096] f32, U [4096, 512], V [4096, 512], s/mask [512].

Strategy: data-parallel over the batch dim (one batch element per core).
Host folds diag(s*mask) into U (U_s = U * s_masked), pre-transposes x to
feature-major, and casts x/U_s/V.T to fp16 packed in SBUF partition
layouts -- so the device kernel is two back-to-back GEMM streams with
zero on-chip transposes:

  GEMM1: tT[k, tok]  += U_s[feat, k].T  @ xT[feat, tok]   (contract feat)
  GEMM2: out[tok, o] += tT[k, tok].T    @ Vt[k, o]        (contract k)

Both run fp16 (1 cycle/row on the PE, f32 PSUM accumulate). PE floor:
1024 MMs x 512 cols ~ 221us/core. GEMM1 iterates feature-piece-major
with one live PSUM bank per rank tile, and us/x arrive as 512KB pieces
in exact consumption order on separate rings, so the PE starts ~12us in
and trickles behind the DMA. V.T is sequenced behind chunk 1's x (it
isn't needed until ~75us). Output stores alternate between the scalar
and sync HWDGE rings as 512KB quarters -- one ring can't sustain the
tail-phase store rate (per-DMA ~2us completion receipt is serialized by
the ring FIFO and backs up into the DVE eviction queue).
"""

import numpy as np

import concourse.bacc as bacc
import concourse.mybir as mybir
import concourse.tile as tile
from concourse.bass import _add_dep_helper
from concourse.bass_utils import run_bass_kernel_spmd

B, S, IN_F, OUT_F, KR = 8, 2048, 4096, 4096, 512
P = 128
N_CORES = 8
FT = IN_F // P  # 32 feature tiles (contraction of GEMM1)
MT = KR // P  # 4 rank tiles (contraction of GEMM2)
CW = 512  # token chunk width (moving free dim of GEMM1)
CH = S // CW  # 4 chunks
TS = CW // P  # 4 token subtiles per chunk (GEMM2 stationary)
OW = 512  # out-feature slice width (moving free dim of GEMM2)
OC = OUT_F // OW  # 8 slices
FP = 8  # x-chunk DMA pieces / GEMM1 f-piece groups
FS = FT // FP  # 4 feature tiles per piece
OQ = 1024  # output store quarter width (2 oc slices)

F32 = mybir.dt.float32
F16 = mybir.dt.float16
NP_F16 = np.float16


def build():
    nc = bacc.Bacc()
    # x pre-transposed + chunked on host: [chunk, 128 feat_p, 32 f, 512 tok]
    x_d = nc.declare_dram_parameter("xt", [CH, P, FT, CW], F16, isOutput=False)
    # U*s stationary tiles, f-piece-major: [128 feat_p, 8 fp, 4 m, 4 fi, 128 k]
    us_d = nc.declare_dram_parameter("us", [P, FP, MT, FS, P], F16, isOutput=False)
    # V.T in moving layout [128 k_p, 4 m, 4096 o]
    vt_d = nc.declare_dram_parameter("vt", [P, MT, OUT_F], F16, isOutput=False)
    out_d = nc.declare_dram_parameter("out", [S, OUT_F], F32, isOutput=True)

    with tile.TileContext(nc) as tc:
        with (
            tc.tile_pool(name="wpool", bufs=1) as wpool,
            tc.tile_pool(name="xc", bufs=2) as xc_p,
            tc.tile_pool(name="tt", bufs=2) as tt_p,
            tc.tile_pool(name="ost", bufs=8) as ost_p,
            tc.tile_pool(name="ps1", bufs=4, space="PSUM") as ps1,
            tc.tile_pool(name="ps2", bufs=4, space="PSUM") as ps2,
        ):
            # Weights resident all kernel. us rides the scalar HWDGE ring
            # as 8x 512KB pieces in f-piece order -- GEMM1 consumes piece
            # fp fully before needing fp+1. vt rides gpsimd but is held
            # behind chunk 1's x stream: the early HBM window belongs to
            # us + x.
            us_t = wpool.tile([P, FP, MT, FS, P], F16)
            vt_t = wpool.tile([P, MT, OUT_F], F16)
            for fp in range(FP):
                nc.scalar.dma_start(us_t[:, fp], us_d[:, fp])
            vt_dmas = [
                nc.gpsimd.dma_start(vt_t[:, m, :], vt_d[:, m, :]) for m in range(MT)
            ]

            # x chunk DMAs, split into f-pieces so GEMM1 can start on
            # piece 0 while later pieces stream.
            xcs = {}
            xdmas = {}

            def fetch_chunk(c):
                xct = xc_p.tile([P, FT, CW], F16, tag="xc")
                for fp in range(FP):
                    xdmas[(c, fp)] = nc.sync.dma_start(
                        xct[:, fp * FS : (fp + 1) * FS, :],
                        x_d[c, :, fp * FS : (fp + 1) * FS, :],
                    )
                xcs[c] = xct

            def gemm1(c):
                # f-piece-major with one live PSUM bank per rank tile m:
                # each 512KB x piece is consumed by 16 matmuls (~3.5us),
                # so the PE trickles behind the chunk DMA instead of
                # stalling for the full 4MB.
                xct = xcs.pop(c)
                tt_c = tt_p.tile([P, MT, CW], F16, tag="tt")
                banks = [
                    ps1.tile([P, CW], F32, tag="p1", name=f"p1_{m}")
                    for m in range(MT)
                ]
                for fp in range(FP):
                    for m in range(MT):
                        for fi in range(FS):
                            f = fp * FS + fi
                            nc.tensor.matmul(
                                banks[m][:],
                                us_t[:, fp, m, fi, :],
                                xct[:, f, :],
                                start=(f == 0),
                                stop=(f == FT - 1),
                            )
                for m in range(MT):
                    # all GEMM1 evictions on ACT: GEMM1's PSUM-bank reuse
                    # deps then never thread through GEMM2's (DVE) queue
                    nc.scalar.copy(tt_c[:, m, :], banks[m][:])
                return tt_c

            def gemm2(c, tt_c):
                for ts in range(TS):
                    tok0 = c * CW + ts * P
                    for q in range(OC // 2):
                        ost = ost_p.tile([P, OQ], F32, tag="ost")
                        for oh in range(2):
                            oc = q * 2 + oh
                            p2 = ps2.tile([P, OW], F32, tag="p2")
                            for m in range(MT):
                                nc.tensor.matmul(
                                    p2[:],
                                    tt_c[:, m, ts * P : (ts + 1) * P],
                                    vt_t[:, m, oc * OW : (oc + 1) * OW],
                                    start=(m == 0),
                                    stop=(m == MT - 1),
                                )
                            # all GEMM2 evictions on DVE (see gemm1)
                            nc.vector.tensor_copy(
                                ost[:, oh * OW : (oh + 1) * OW], p2[:]
                            )
                        # alternate store ring: one HWDGE ring can't keep
                        # up with the tail-phase store rate
                        ring = nc.scalar if (ts * (OC // 2) + q) % 2 == 0 else nc.sync
                        ring.dma_start(
                            out_d[tok0 : tok0 + P, q * OQ : (q + 1) * OQ],
                            ost[:],
                        )

            # GEMM2 skewed one chunk behind GEMM1 so tt evictions and vt
            # streaming have a full phase of slack before the PE needs them.
            fetch_chunk(0)
            fetch_chunk(1)
            # vt yields the early HBM window to us + x chunks 0/1
            for m, vdma in enumerate(vt_dmas):
                _add_dep_helper(
                    vdma.ins,
                    xdmas[(1, 2 * m)].ins,
                    sync=True,
                    reason="stagger vt loads behind early x stream",
                )
            tts = {}
            for c in range(CH + 1):
                if c < CH:
                    tts[c] = gemm1(c)
                    if c + 2 < CH:
                        fetch_chunk(c + 2)
                if c >= 1:
                    gemm2(c - 1, tts.pop(c - 1))
    nc.finalize()
    return nc


_NC_CACHE = {}


def _get_nc():
    key = "main"
    if key not in _NC_CACHE:
        _NC_CACHE[key] = build()
    return _NC_CACHE[key]


def kernel(x, U, V, s, mask, _trace=False, _trace_kwargs=None):
    x = np.asarray(x)
    U = np.asarray(U)
    V = np.asarray(V)
    s = np.asarray(s)
    mask = np.asarray(mask)
    s_masked = (s.astype(np.float32) * mask.astype(np.float32)).astype(np.float32)
    U_s = U.astype(np.float32) * s_masked[None, :]
    # us[p, fp, m, fi, kk] = U_s[(fp*FS+fi)*128+p, m*128+kk]
    us_prep = np.ascontiguousarray(
        U_s.reshape(FP, FS, P, MT, P).transpose(2, 0, 3, 1, 4).astype(NP_F16)
    )
    # vt[p, m, o] = V.T[m*128+p, o] = V[o, m*128+p]
    vt_prep = np.ascontiguousarray(
        V.astype(np.float32).T.reshape(MT, P, OUT_F).transpose(1, 0, 2).astype(NP_F16)
    )
    nc = _get_nc()
    in_maps = []
    for b in range(B):
        # xt[c, p, f, t] = x[b, c*CW+t, f*128+p]
        xt = np.ascontiguousarray(
            x[b].reshape(CH, CW, FT, P).transpose(0, 3, 2, 1).astype(NP_F16)
        )
        in_maps.append({"xt": xt, "us": us_prep, "vt": vt_prep})
    res = run_bass_kernel_spmd(
        nc, in_maps, list(range(N_CORES)), trace=_trace, **(_trace_kwargs or {})
    )
    out = np.stack([res.results[b]["out"] for b in range(B)], axis=0)
    if _trace:
        return out, res
    return out


# revision 10
# speedup vs baseline: 1.1852x; 1.1852x over previous
"""DLoRF low-rank linear kernel for Trainium2 (8 NeuronCores, SPMD).

Computes  out = x @ U @ diag(s * mask) @ V.T  for
  x [8, 2048, 4096] f32, U [4096, 512], V [4096, 512], s/mask [512].

Strategy: data-parallel over the batch dim (one batch element per core).
Host folds diag(s*mask) into U (U_s = U * s_masked), pre-transposes x to
feature-major, and casts x/U_s/V.T to fp16 packed in SBUF partition
layouts -- so the device kernel is two back-to-back GEMM streams with
zero on-chip transposes:

  GEMM1: tT[k, tok]  += U_s[feat, k].T  @ xT[feat, tok]   (contract feat)
  GEMM2: out[tok, o] += tT[k, tok].T    @ Vt[k, o]        (contract k)

Both run fp16 (1 cycle/row on the PE, f32 PSUM accumulate). PE floor:
1024 MMs x 512 cols ~ 221us/core. GEMM1 iterates feature-piece-major
with one live PSUM bank per rank tile, and us/x arrive as 512KB pieces
in exact consumption order on separate rings, so the PE starts ~12us in
and trickles behind the DMA. V.T is sequenced behind chunk 1's x (it
isn't needed until ~75us). Output stores alternate between the scalar
and sync HWDGE rings as 512KB quarters -- one ring can't sustain the
tail-phase store rate (per-DMA ~2us completion receipt is serialized by
the ring FIFO and backs up into the DVE eviction queue).
"""

import numpy as np

import concourse.bacc as bacc
import concourse.mybir as mybir
import concourse.tile as tile
from concourse.bass import _add_dep_helper
from concourse.bass_utils import run_bass_kernel_spmd

B, S, IN_F, OUT_F, KR = 8, 2048, 4096, 4096, 512
P = 128
N_CORES = 8
FT = IN_F // P  # 32 feature tiles (contraction of GEMM1)
MT = KR // P  # 4 rank tiles (contraction of GEMM2)
CW = 512  # token chunk width (moving free dim of GEMM1)
CH = S // CW  # 4 chunks
TS = CW // P  # 4 token subtiles per chunk (GEMM2 stationary)
OW = 512  # out-feature slice width (moving free dim of GEMM2)
OC = OUT_F // OW  # 8 slices
FP = 8  # x-chunk DMA pieces / GEMM1 f-piece groups
FS = FT // FP  # 4 feature tiles per piece
OQ = 1024  # output store quarter width (2 oc slices)

F32 = mybir.dt.float32
F16 = mybir.dt.float16
NP_F16 = np.float16


def build():
    nc = bacc.Bacc()
    # x pre-transposed + chunked on host: [chunk, 128 feat_p, 32 f, 512 tok]
    x_d = nc.declare_dram_parameter("xt", [CH, P, FT, CW], F16, isOutput=False)
    # U*s stationary tiles, f-piece-major: [128 feat_p, 8 fp, 4 m, 4 fi, 128 k]
    us_d = nc.declare_dram_parameter("us", [P, FP, MT, FS, P], F16, isOutput=False)
    # V.T in moving layout [128 k_p, 4 m, 4096 o]
    vt_d = nc.declare_dram_parameter("vt", [P, MT, OUT_F], F16, isOutput=False)
    out_d = nc.declare_dram_parameter("out", [S, OUT_F], F32, isOutput=True)

    with tile.TileContext(nc) as tc:
        with (
            tc.tile_pool(name="wpool", bufs=1) as wpool,
            tc.tile_pool(name="xc", bufs=2) as xc_p,
            tc.tile_pool(name="tt", bufs=2) as tt_p,
            tc.tile_pool(name="ost", bufs=10) as ost_p,
            tc.tile_pool(name="ps1", bufs=4, space="PSUM") as ps1,
            tc.tile_pool(name="ps2", bufs=4, space="PSUM") as ps2,
        ):
            # Weights resident all kernel. us rides the scalar HWDGE ring
            # as 8x 512KB pieces in f-piece order -- GEMM1 consumes piece
            # fp fully before needing fp+1. vt rides gpsimd but is held
            # behind chunk 1's x stream: the early HBM window belongs to
            # us + x.
            us_t = wpool.tile([P, FP, MT, FS, P], F16)
            vt_t = wpool.tile([P, MT, OUT_F], F16)
            for fp in range(FP):
                nc.scalar.dma_start(us_t[:, fp], us_d[:, fp])
            vt_dmas = [
                nc.gpsimd.dma_start(vt_t[:, m, :], vt_d[:, m, :]) for m in range(MT)
            ]

            # x chunk DMAs, split into f-pieces so GEMM1 can start on
            # piece 0 while later pieces stream.
            xcs = {}
            xdmas = {}

            def fetch_chunk(c):
                xct = xc_p.tile([P, FT, CW], F16, tag="xc")
                for fp in range(FP):
                    xdmas[(c, fp)] = nc.sync.dma_start(
                        xct[:, fp * FS : (fp + 1) * FS, :],
                        x_d[c, :, fp * FS : (fp + 1) * FS, :],
                    )
                xcs[c] = xct

            def gemm1(c):
                # f-piece-major with one live PSUM bank per rank tile m:
                # each 512KB x piece is consumed by 16 matmuls (~3.5us),
                # so the PE trickles behind the chunk DMA instead of
                # stalling for the full 4MB.
                xct = xcs.pop(c)
                tt_c = tt_p.tile([P, MT, CW], F16, tag="tt")
                banks = [
                    ps1.tile([P, CW], F32, tag="p1", name=f"p1_{m}")
                    for m in range(MT)
                ]
                for fp in range(FP):
                    for m in range(MT):
                        for fi in range(FS):
                            f = fp * FS + fi
                            nc.tensor.matmul(
                                banks[m][:],
                                us_t[:, fp, m, fi, :],
                                xct[:, f, :],
                                start=(f == 0),
                                stop=(f == FT - 1),
                            )
                for m in range(MT):
                    # alternate eviction engine so neither ACT nor DVE
                    # gates PSUM recycling
                    copy_eng = nc.scalar.copy if m % 2 == 0 else nc.vector.tensor_copy
                    copy_eng(tt_c[:, m, :], banks[m][:])
                return tt_c

            def gemm2(c, tt_c):
                for ts in range(TS):
                    tok0 = c * CW + ts * P
                    for q in range(OC // 2):
                        ost = ost_p.tile([P, OQ], F32, tag="ost")
                        for oh in range(2):
                            oc = q * 2 + oh
                            p2 = ps2.tile([P, OW], F32, tag="p2")
                            for m in range(MT):
                                nc.tensor.matmul(
                                    p2[:],
                                    tt_c[:, m, ts * P : (ts + 1) * P],
                                    vt_t[:, m, oc * OW : (oc + 1) * OW],
                                    start=(m == 0),
                                    stop=(m == MT - 1),
                                )
                            copy_eng = (
                                nc.scalar.copy if oh == 0 else nc.vector.tensor_copy
                            )
                            copy_eng(ost[:, oh * OW : (oh + 1) * OW], p2[:])
                        # alternate store ring: one HWDGE ring can't keep
                        # up with the tail-phase store rate
                        ring = nc.scalar if (ts * (OC // 2) + q) % 2 == 0 else nc.sync
                        ring.dma_start(
                            out_d[tok0 : tok0 + P, q * OQ : (q + 1) * OQ],
                            ost[:],
                        )

            # GEMM2 skewed one chunk behind GEMM1 so tt evictions and vt
            # streaming have a full phase of slack before the PE needs them.
            fetch_chunk(0)
            fetch_chunk(1)
            # vt yields the early HBM window to us + x chunks 0/1
            for m, vdma in enumerate(vt_dmas):
                _add_dep_helper(
                    vdma.ins,
                    xdmas[(1, 2 * m)].ins,
                    sync=True,
                    reason="stagger vt loads behind early x stream",
                )
            tts = {}
            for c in range(CH + 1):
                if c < CH:
                    tts[c] = gemm1(c)
                    if c + 2 < CH:
                        fetch_chunk(c + 2)
                if c >= 1:
                    gemm2(c - 1, tts.pop(c - 1))
    nc.finalize()
    return nc


_NC_CACHE = {}


def _get_nc():
    key = "main"
    if key not in _NC_CACHE:
        _NC_CACHE[key] = build()
    return _NC_CACHE[key]


def kernel(x, U, V, s, mask, _trace=False, _trace_kwargs=None):
    x = np.asarray(x)
    U = np.asarray(U)
    V = np.asarray(V)
    s = np.asarray(s)
    mask = np.asarray(mask)
    s_masked = (s.astype(np.float32) * mask.astype(np.float32)).astype(np.float32)
    U_s = U.astype(np.float32) * s_masked[None, :]
    # us[p, fp, m, fi, kk] = U_s[(fp*FS+fi)*128+p, m*128+kk]
    us_prep = np.ascontiguousarray(
        U_s.reshape(FP, FS, P, MT, P).transpose(2, 0, 3, 1, 4).astype(NP_F16)
    )
    # vt[p, m, o] = V.T[m*128+p, o] = V[o, m*128+p]
    vt_prep = np.ascontiguousarray(
        V.astype(np.float32).T.reshape(MT, P, OUT_F).transpose(1, 0, 2).astype(NP_F16)
    )
    nc = _get_nc()
    in_maps = []
    for b in range(B):
        # xt[c, p, f, t] = x[b, c*CW+t, f*128+p]
        xt = np.ascontiguousarray(
            x[b].reshape(CH, CW, FT, P).transpose(0, 3, 2, 1).astype(NP_F16)
        )
        in_maps.append({"xt": xt, "us": us_prep, "vt": vt_prep})
    res = run_bass_kernel_spmd(
        nc, in_maps, list(range(N_CORES)), trace=_trace, **(_trace_kwargs or {})
    )
    out = np.stack([res.results[b]["out"] for b in range(B)], axis=0)
    if _trace:
        return out, res
    return out
